# revision 21
# baseline (speedup 1.0000x reference)
"""GenerativeInfoNCE loss on 8 Trainium2 NeuronCores (Bass/Tile).

Strategy (data-parallel over batch, per the sharding hint):
  - Shard the 32 batches across 8 cores (4 batches / core -> 2044 rows of
    (b, s) prediction positions, padded to 2048 = 16 tiles of 128).
  - Every core gets the FULL event table (16384 x 1024, bf16) in its HBM.
  - Negatives are fetched with the batched `dma_gather` extended
    instruction (InstDMAGatherAnt): ONE instruction gathers all 1280 rows
    (128 rows x 10 negatives) of a tile. Its Q7 ucode emits descriptors
    via the CounterMachine across 16 SDMA engines -- orders of magnitude
    cheaper than per-row indirect DMA descriptor generation (which costs
    ~1 us/descriptor and made the original version DMA-bound at ~23 ms).
    Probed limits: >64 data descriptors per SDMA ring must NOT coalesce
    into one packet (single_packet=False), and the gather path is Q7
    descriptor-generation bound (~7 ns/descriptor), flat in elem_size.
  - Dots: 10 scalar_tensor_tensor instructions per 128-row tile on the
    Vector engine (fp32 accumulate; measured 310 ns + 1.24 ns/elem). A
    fused broadcast-TT + chunked-reduce variant (fused=True) measured
    equal-or-slower on HW (the broadcast source loses the packed 2x DVE
    mode), so STT is the default. |p|^2 / |q|^2 row norms run on the
    otherwise-idle Scalar engine (Square activation with accumulate).
    The positive-score epilogue (sqrt / reciprocal / scale) is batched
    over all 16 tiles.
  - logsumexp over the 11 logits is batched once at the end; per-row
    losses are DMA'd out and the host does the final mean in float64
    (the "psum-mean" step of the hint, done on host since it is 16K
    scalars).

dma_gather layout contract (from bass_interp + dma_gather.cpp ucode):
  - gathered vector i lands at dst[i % 128, i // 128, :]; we order the
    flat index list as i = j*128 + p so dst[p, j, :] = events[idx[p, j]].
  - the int16 index tile is read as [16, num_idxs/16] from the Q7
    core-pair's native partition window: index i at partition i % 16,
    column i // 16. We replicate the block across all eight 16-partition
    groups (the docstring's "replicated across cores").
  - indices fit int16 (max 16383 < 32767).

The index remap (skip own batch's block of S events) is pure int math and
is done on the host in numpy before sharding.
"""

import numpy as np

import concourse.bacc as bacc
import concourse.bass as bass
import concourse.tile as tile
from concourse import mybir
from concourse.bass_utils import run_bass_kernel_spmd

B, S, H, NEG = 32, 512, 1024, 10
NCORES = 8
BPC = B // NCORES            # batches per core
R = BPC * (S - 1)            # valid rows per core (2044)
NT = (R + 127) // 128        # 16 tiles of 128 rows
RP = NT * 128                # padded rows (2048)
K = NEG + 1                  # logits per row
NIDX = 128 * NEG             # indices gathered per tile (1280)
ICOLS = NIDX // 16           # int16 index columns per tile (80)
CH = 32                      # stage-1 reduce chunk width

BF16 = mybir.dt.bfloat16
F32 = mybir.dt.float32
I16 = mybir.dt.int16

NQ = 4  # SWDGE queues; round-robin measured 1.43x on the gather path
# Dots per tile computed on the Scalar engine via the polarization
# identity p.g = (|p+g|^2 - |p|^2 - |g|^2)/2 instead of a DVE
# scalar_tensor_tensor: DVE pays only a bf16 tensor_tensor add (2x packed
# mode, 632 ns measured); ACT (otherwise idle) does the |p+g|^2
# square+accumulate (1277 ns). The |g|^2 event-row norms are a pure
# function of the inputs, precomputed once on the host (like the index
# remap) and fed per (row, neg) as an f32 input — so ACT pays ONE square
# per offloaded dot, letting it absorb 7 of the 10. Balances DVE
# (~11.6 us/tile) vs ACT (~11.5 us/tile).
ACT_DOTS = 7


def _build(temp: float, reps: int = 1, no_gather: bool = False,
           no_dot: bool = False, fused: bool = False, nq: int = NQ):
    """Build + compile the per-core program (identical on all 8 cores).

    reps > 1 wraps the whole body in a hardware For loop (used only for
    timing; the work is identical every iteration). no_gather / no_dot
    ablate the negative-gather DMAs / the DVE dot products (timing
    experiments only — results are wrong with either set; they force
    fused=False).
    """
    if no_gather or no_dot:
        fused = False
    nc = bacc.Bacc("TRN2", target_bir_lowering=False, debug=False,
                   num_devices=NCORES, num_swdge_queues=nq)

    ev_d = nc.dram_tensor("events", [B * S, H], BF16, kind="ExternalInput")
    pred_d = nc.dram_tensor("pred", [RP, H], BF16, kind="ExternalInput")
    pos_d = nc.dram_tensor("pos", [RP, H], BF16, kind="ExternalInput")
    idx_d = nc.dram_tensor("idx", [128, NT * ICOLS], I16, kind="ExternalInput")
    gn2_d = nc.dram_tensor("gn2", [128, NT * NEG], F32, kind="ExternalInput")
    out_d = nc.dram_tensor("loss", [128, NT], F32, kind="ExternalOutput")

    inv_t = 1.0 / temp
    mult = mybir.AluOpType.mult
    add = mybir.AluOpType.add
    AF = mybir.ActivationFunctionType
    X = mybir.AxisListType.X

    with tile.TileContext(nc) as tc:
        import contextlib
        with contextlib.ExitStack() as ctx:
            io = ctx.enter_context(tc.tile_pool(name="io", bufs=4))
            gp = ctx.enter_context(tc.tile_pool(name="gather", bufs=3))
            fp = ctx.enter_context(tc.tile_pool(name="prod", bufs=2))
            sup = ctx.enter_context(tc.tile_pool(name="sums", bufs=4))
            scrp = ctx.enter_context(tc.tile_pool(name="scratch", bufs=2))
            sap = ctx.enter_context(tc.tile_pool(name="actscr", bufs=2))
            sm = ctx.enter_context(tc.tile_pool(name="small", bufs=8))
            pers = ctx.enter_context(tc.tile_pool(name="persist", bufs=1))

            loop_cm = tc.For_i(0, reps, 1) if reps > 1 else None
            if loop_cm is not None:
                ctx.enter_context(loop_cm)

            idx_t = pers.tile([128, NT * ICOLS], I16, tag="idx")
            nc.sync.dma_start(out=idx_t[:], in_=idx_d.ap())
            gn2_t = pers.tile([128, NT * NEG], F32, tag="gn2")
            nc.sync.dma_start(out=gn2_t[:], in_=gn2_d.ap())
            logits = pers.tile([128, NT * K], F32, tag="logits")
            pn2s = pers.tile([128, NT], F32, tag="pn2s")
            qn2s = pers.tile([128, NT], F32, tag="qn2s")
            ppds = pers.tile([128, NT], F32, tag="ppds")

            for t in range(NT):
                rs = slice(t * 128, (t + 1) * 128)
                pred_t = io.tile([128, H], BF16, tag="pred")
                nc.sync.dma_start(out=pred_t[:], in_=pred_d.ap()[rs, :])
                pos_t = io.tile([128, H], BF16, tag="pos")
                nc.sync.dma_start(out=pos_t[:], in_=pos_d.ap()[rs, :])

                # One batched gather for all 10 negatives of the tile's 128
                # rows: g[p, j, :] = events[idx[p, j]] (flat order j*128+p).
                g = None
                if not no_gather:
                    g = gp.tile([128, NEG, H], BF16, tag="g")
                    nc.gpsimd.dma_gather(
                        out_ap=g[:],
                        in_ap=ev_d.ap(),
                        idxs_ap=idx_t[:, t * ICOLS:(t + 1) * ICOLS],
                        num_idxs=NIDX,
                        num_idxs_reg=NIDX,
                        elem_size=H,
                        # >64 data descriptors per SDMA ring cannot legally
                        # coalesce into one packet; 1280 idxs = 80/ring.
                        single_packet=False,
                        queue_num=t % nq,
                    )

                scr = scrp.tile([128, H], BF16, tag="scr")
                if fused:
                    prod = fp.tile([128, NEG, H], BF16, tag="prod")
                    nc.vector.tensor_tensor(
                        out=prod[:],
                        in0=pred_t[:].rearrange("p (o h) -> p o h", o=1)
                                     .to_broadcast([128, NEG, H]),
                        in1=g[:], op=mult)
                    red1 = fp.tile([128, NEG, H // CH], BF16, tag="red1")
                    with nc.allow_low_precision(
                            reason="bf16 partial sums of 32 products; "
                                   "validated: final loss rel err ~1e-4"):
                        nc.vector.tensor_reduce(
                            out=red1[:],
                            in_=prod[:].rearrange("p n (c k) -> p n c k",
                                                  k=CH),
                            axis=X, op=add)
                    negf = sm.tile([128, NEG], F32, tag="negf")
                    nc.vector.tensor_reduce(
                        out=negf[:], in_=red1[:], axis=X, op=add)
                    nc.vector.tensor_scalar_mul(
                        out=logits[:, t * K + 1:(t + 1) * K], in0=negf[:],
                        scalar1=inv_t)
                elif not no_dot:
                    koff = ACT_DOTS if g is not None else 0
                    ndve = NEG - koff
                    for j in range(ndve):
                        c = t * K + 1 + j
                        in1 = g[:, j, :] if g is not None else pred_t[:]
                        nc.vector.scalar_tensor_tensor(
                            out=scr[:], in0=pred_t[:], scalar=inv_t,
                            in1=in1, op0=mult, op1=mult,
                            accum_out=logits[:, c:c + 1],
                        )
                    if koff:
                        sqs = sm.tile([128, koff], F32, tag="sqs")
                        for jj in range(koff):
                            j = ndve + jj
                            su = sup.tile([128, H], BF16, tag="su")
                            nc.vector.tensor_add(
                                out=su[:], in0=pred_t[:], in1=g[:, j, :])
                            scs = sap.tile([128, H], BF16, tag="scs")
                            nc.scalar.activation(
                                out=scs[:], in_=su[:], func=AF.Square,
                                accum_out=sqs[:, jj:jj + 1])
                else:
                    nc.vector.memset(logits[:, t * K + 1:(t + 1) * K], 1.0)

                # row norms on the Scalar engine (Square + accumulate);
                # pred.pos dot on DVE (scalar_tensor_tensor, fp32 accum).
                scra = sap.tile([128, H], BF16, tag="scra")
                nc.scalar.activation(
                    out=scra[:], in_=pred_t[:], func=AF.Square,
                    accum_out=pn2s[:, t:t + 1])
                scrb = sap.tile([128, H], BF16, tag="scrb")
                nc.scalar.activation(
                    out=scrb[:], in_=pos_t[:], func=AF.Square,
                    accum_out=qn2s[:, t:t + 1])
                nc.vector.scalar_tensor_tensor(
                    out=scr[:], in0=pred_t[:], scalar=inv_t, in1=pos_t[:],
                    op0=mult, op1=mult, accum_out=ppds[:, t:t + 1])

                if (not fused) and (not no_dot) and g is not None and ACT_DOTS:
                    # assemble the ACT-identity dots now that pn2 is ready:
                    # logit_j = (|p+g_j|^2 - |g_j|^2 - |p|^2) / (2T),
                    # with |g_j|^2 precomputed on host and gathered per row.
                    koff = ACT_DOTS
                    ndve = NEG - koff
                    cmb = sm.tile([128, koff], F32, tag="cmb")
                    nc.vector.tensor_sub(
                        out=cmb[:], in0=sqs[:],
                        in1=gn2_t[:, t * NEG + ndve:(t + 1) * NEG])
                    nc.vector.tensor_sub(
                        out=cmb[:], in0=cmb[:],
                        in1=pn2s[:, t:t + 1].to_broadcast([128, koff]))
                    nc.vector.tensor_scalar_mul(
                        out=logits[:, t * K + 1 + ndve:(t + 1) * K],
                        in0=cmb[:], scalar1=0.5 * inv_t)

            # Batched positive-logit epilogue over all NT tiles:
            # logits[:, t*K] = (ppd/T) / sqrt(pn2*qn2)
            l3 = logits[:].rearrange("p (t k) -> p t k", k=K)
            nrm = pers.tile([128, NT], F32, tag="nrm")
            nc.vector.tensor_mul(out=nrm[:], in0=pn2s[:], in1=qn2s[:])
            nc.scalar.activation(out=nrm[:], in_=nrm[:], func=AF.Sqrt)
            rn = pers.tile([128, NT], F32, tag="rn")
            nc.vector.reciprocal(out=rn[:], in_=nrm[:])
            nc.vector.tensor_mul(
                out=l3[:, :, 0:1],
                in0=ppds[:].rearrange("p (t o) -> p t o", o=1),
                in1=rn[:].rearrange("p (t o) -> p t o", o=1))

            # Batched logsumexp over all NT tiles at once.
            m = pers.tile([128, NT, 1], F32, tag="m")
            nc.vector.reduce_max(out=m[:], in_=l3, axis=X)
            sh = pers.tile([128, NT, K], F32, tag="sh")
            nc.vector.tensor_sub(out=sh[:], in0=l3, in1=m[:].to_broadcast([128, NT, K]))
            eh = pers.tile([128, NT, K], F32, tag="eh")
            nc.scalar.activation(out=eh[:], in_=sh[:], func=AF.Exp)
            ss = pers.tile([128, NT, 1], F32, tag="ss")
            nc.vector.reduce_sum(out=ss[:], in_=eh[:], axis=X)
            nc.scalar.activation(out=ss[:], in_=ss[:], func=AF.Ln)
            outt = pers.tile([128, NT], F32, tag="outt")
            nc.vector.tensor_add(out=outt[:], in0=m[:, :, 0], in1=ss[:, :, 0])
            nc.vector.tensor_sub(out=outt[:], in0=outt[:], in1=l3[:, :, 0])
            nc.sync.dma_start(out=out_d.ap(), in_=outt[:])

    nc.compile()
    return nc


def _prep_in_maps(encoder_outputs, event_embeddings, neg_indices):
    enc = np.asarray(encoder_outputs, dtype=np.float32)
    ev = np.asarray(event_embeddings, dtype=np.float32)
    ni = np.asarray(neg_indices)
    bf = mybir.dt.np(BF16)

    b_ids = np.arange(B, dtype=ni.dtype)[:, None, None]
    gidx = (ni + S * (ni >= b_ids * S).astype(ni.dtype)).astype(np.int32)

    ev_flat = np.ascontiguousarray(ev.reshape(B * S, H)).astype(bf)
    # |e|^2 per event row, from the same bf16-cast table the device dots
    # against (pure function of the inputs, like the index remap).
    ev_n2 = (ev_flat.astype(np.float32) ** 2).sum(-1)  # [B*S]

    in_maps = []
    for c in range(NCORES):
        bs = slice(c * BPC, (c + 1) * BPC)
        pred = enc[bs, :-1, :].reshape(R, H)
        pos = ev[bs, 1:, :].reshape(R, H)
        pred_p = np.ones((RP, H), np.float32)
        pred_p[:R] = pred
        pos_p = np.ones((RP, H), np.float32)
        pos_p[:R] = pos
        idx = np.zeros((RP, NEG), np.int32)
        idx[:R] = gidx[bs].reshape(R, NEG)
        # dma_gather index layout: per tile t the flat gather order is
        # i = j*128 + p; index i lives at partition i%16, column i//16.
        tiles = idx.reshape(NT, 128, NEG).transpose(0, 2, 1).reshape(NT, NIDX)
        arr = tiles.reshape(NT, ICOLS, 16).transpose(0, 2, 1)  # [NT, 16, ICOLS]
        band = arr.transpose(1, 0, 2).reshape(16, NT * ICOLS).astype(np.int16)
        dev = np.tile(band, (8, 1))  # replicate to all 8 Q7 partition groups
        # gathered |g_j|^2 per (row, neg): device layout [128, NT*NEG],
        # partition = row-within-tile, column = t*NEG + j
        gn2 = ev_n2[idx]  # [RP, NEG]
        gn2_dev = np.ascontiguousarray(
            gn2.reshape(NT, 128, NEG).transpose(1, 0, 2)
        ).reshape(128, NT * NEG).astype(np.float32)
        in_maps.append({
            "events": ev_flat,
            "pred": pred_p.astype(bf),
            "pos": pos_p.astype(bf),
            "idx": dev,
            "gn2": gn2_dev,
        })
    return in_maps


def _reduce_loss(results) -> np.float32:
    total = 0.0
    for c in range(NCORES):
        lr = np.asarray(results[c]["loss"], dtype=np.float64)  # [128, NT]
        rows = lr.T.reshape(RP)[:R]
        total += rows.sum()
    return np.float32(total / (B * (S - 1)))


_NC_CACHE: dict = {}


def kernel(encoder_outputs, event_embeddings, neg_indices, temperature):
    temp = float(np.asarray(temperature))
    nc = _NC_CACHE.get(temp)
    if nc is None:
        nc = _build(temp)
        _NC_CACHE[temp] = nc
    in_maps = _prep_in_maps(encoder_outputs, event_embeddings, neg_indices)
    res = run_bass_kernel_spmd(nc, in_maps, core_ids=list(range(NCORES)))
    return _reduce_loss(res.results)


# revision 22
# speedup vs baseline: 1.0903x; 1.0903x over previous
"""GenerativeInfoNCE loss on 8 Trainium2 NeuronCores (Bass/Tile).

Strategy (data-parallel over batch, per the sharding hint):
  - Shard the 32 batches across 8 cores (4 batches / core -> 2044 rows of
    (b, s) prediction positions, padded to 2048 = 16 tiles of 128).
  - Every core gets the FULL event table (16384 x 1024, bf16) in its HBM.
  - Negatives are fetched with the batched `dma_gather` extended
    instruction (InstDMAGatherAnt): ONE instruction gathers all 1280 rows
    (128 rows x 10 negatives) of a tile. Its Q7 ucode emits descriptors
    via the CounterMachine across 16 SDMA engines -- orders of magnitude
    cheaper than per-row indirect DMA descriptor generation (which costs
    ~1 us/descriptor and made the original version DMA-bound at ~23 ms).
    Probed limits: >64 data descriptors per SDMA ring must NOT coalesce
    into one packet (single_packet=False), and the gather path is Q7
    descriptor-generation bound (~7 ns/descriptor), flat in elem_size.
  - Dots: 10 scalar_tensor_tensor instructions per 128-row tile on the
    Vector engine (fp32 accumulate; measured 310 ns + 1.24 ns/elem). A
    fused broadcast-TT + chunked-reduce variant (fused=True) measured
    equal-or-slower on HW (the broadcast source loses the packed 2x DVE
    mode), so STT is the default. |p|^2 / |q|^2 row norms run on the
    otherwise-idle Scalar engine (Square activation with accumulate).
    The positive-score epilogue (sqrt / reciprocal / scale) is batched
    over all 16 tiles.
  - logsumexp over the 11 logits is batched once at the end; per-row
    losses are DMA'd out and the host does the final mean in float64
    (the "psum-mean" step of the hint, done on host since it is 16K
    scalars).

dma_gather layout contract (from bass_interp + dma_gather.cpp ucode):
  - gathered vector i lands at dst[i % 128, i // 128, :]; we order the
    flat index list as i = j*128 + p so dst[p, j, :] = events[idx[p, j]].
  - the int16 index tile is read as [16, num_idxs/16] from the Q7
    core-pair's native partition window: index i at partition i % 16,
    column i // 16. We replicate the block across all eight 16-partition
    groups (the docstring's "replicated across cores").
  - indices fit int16 (max 16383 < 32767).

The index remap (skip own batch's block of S events) is pure int math and
is done on the host in numpy before sharding.
"""

import numpy as np

import concourse.bacc as bacc
import concourse.bass as bass
import concourse.tile as tile
from concourse import mybir
from concourse.bass_utils import run_bass_kernel_spmd

B, S, H, NEG = 32, 512, 1024, 10
NCORES = 8
BPC = B // NCORES            # batches per core
R = BPC * (S - 1)            # valid rows per core (2044)
NT = (R + 127) // 128        # 16 tiles of 128 rows
RP = NT * 128                # padded rows (2048)
K = NEG + 1                  # logits per row
NIDX = 128 * NEG             # indices gathered per tile (1280)
ICOLS = NIDX // 16           # int16 index columns per tile (80)
CH = 32                      # stage-1 reduce chunk width

BF16 = mybir.dt.bfloat16
F32 = mybir.dt.float32
I16 = mybir.dt.int16

NQ = 4  # SWDGE queues; round-robin measured 1.43x on the gather path
# Dots per tile computed on the Scalar engine via the polarization
# identity p.g = (|p+g|^2 - |p|^2 - |g|^2)/2 instead of a DVE
# scalar_tensor_tensor: DVE pays only a bf16 tensor_tensor add (2x packed
# mode, 632 ns measured); ACT (otherwise idle) does the |p+g|^2
# square+accumulate (1277 ns). The |g|^2 event-row norms are a pure
# function of the inputs, precomputed once on the host (like the index
# remap) and fed per (row, neg) as an f32 input — so ACT pays ONE square
# per offloaded dot. k=7 measured SLOWER (315us vs 265us): each DVE-fed
# ACT square carries ~0.7us of cross-engine semaphore latency on ACT's
# critical path, so ACT saturates first. k=4 keeps the measured-265us
# dependency graph minus four ACT squares (strictly less work).
ACT_DOTS = 4


def _build(temp: float, reps: int = 1, no_gather: bool = False,
           no_dot: bool = False, fused: bool = False, nq: int = NQ):
    """Build + compile the per-core program (identical on all 8 cores).

    reps > 1 wraps the whole body in a hardware For loop (used only for
    timing; the work is identical every iteration). no_gather / no_dot
    ablate the negative-gather DMAs / the DVE dot products (timing
    experiments only — results are wrong with either set; they force
    fused=False).
    """
    if no_gather or no_dot:
        fused = False
    nc = bacc.Bacc("TRN2", target_bir_lowering=False, debug=False,
                   num_devices=NCORES, num_swdge_queues=nq)

    ev_d = nc.dram_tensor("events", [B * S, H], BF16, kind="ExternalInput")
    pred_d = nc.dram_tensor("pred", [RP, H], BF16, kind="ExternalInput")
    pos_d = nc.dram_tensor("pos", [RP, H], BF16, kind="ExternalInput")
    idx_d = nc.dram_tensor("idx", [128, NT * ICOLS], I16, kind="ExternalInput")
    gn2_d = nc.dram_tensor("gn2", [128, NT * NEG], F32, kind="ExternalInput")
    out_d = nc.dram_tensor("loss", [128, NT], F32, kind="ExternalOutput")

    inv_t = 1.0 / temp
    mult = mybir.AluOpType.mult
    add = mybir.AluOpType.add
    AF = mybir.ActivationFunctionType
    X = mybir.AxisListType.X

    with tile.TileContext(nc) as tc:
        import contextlib
        with contextlib.ExitStack() as ctx:
            io = ctx.enter_context(tc.tile_pool(name="io", bufs=4))
            gp = ctx.enter_context(tc.tile_pool(name="gather", bufs=3))
            fp = ctx.enter_context(tc.tile_pool(name="prod", bufs=2))
            sup = ctx.enter_context(tc.tile_pool(name="sums", bufs=4))
            scrp = ctx.enter_context(tc.tile_pool(name="scratch", bufs=2))
            sap = ctx.enter_context(tc.tile_pool(name="actscr", bufs=2))
            sm = ctx.enter_context(tc.tile_pool(name="small", bufs=8))
            pers = ctx.enter_context(tc.tile_pool(name="persist", bufs=1))

            loop_cm = tc.For_i(0, reps, 1) if reps > 1 else None
            if loop_cm is not None:
                ctx.enter_context(loop_cm)

            idx_t = pers.tile([128, NT * ICOLS], I16, tag="idx")
            nc.sync.dma_start(out=idx_t[:], in_=idx_d.ap())
            gn2_t = pers.tile([128, NT * NEG], F32, tag="gn2")
            nc.sync.dma_start(out=gn2_t[:], in_=gn2_d.ap())
            logits = pers.tile([128, NT * K], F32, tag="logits")
            pn2s = pers.tile([128, NT], F32, tag="pn2s")
            qn2s = pers.tile([128, NT], F32, tag="qn2s")
            ppds = pers.tile([128, NT], F32, tag="ppds")

            for t in range(NT):
                rs = slice(t * 128, (t + 1) * 128)
                pred_t = io.tile([128, H], BF16, tag="pred")
                nc.sync.dma_start(out=pred_t[:], in_=pred_d.ap()[rs, :])
                pos_t = io.tile([128, H], BF16, tag="pos")
                nc.sync.dma_start(out=pos_t[:], in_=pos_d.ap()[rs, :])

                # One batched gather for all 10 negatives of the tile's 128
                # rows: g[p, j, :] = events[idx[p, j]] (flat order j*128+p).
                g = None
                if not no_gather:
                    g = gp.tile([128, NEG, H], BF16, tag="g")
                    nc.gpsimd.dma_gather(
                        out_ap=g[:],
                        in_ap=ev_d.ap(),
                        idxs_ap=idx_t[:, t * ICOLS:(t + 1) * ICOLS],
                        num_idxs=NIDX,
                        num_idxs_reg=NIDX,
                        elem_size=H,
                        # >64 data descriptors per SDMA ring cannot legally
                        # coalesce into one packet; 1280 idxs = 80/ring.
                        single_packet=False,
                        queue_num=t % nq,
                    )

                scr = scrp.tile([128, H], BF16, tag="scr")
                if fused:
                    prod = fp.tile([128, NEG, H], BF16, tag="prod")
                    nc.vector.tensor_tensor(
                        out=prod[:],
                        in0=pred_t[:].rearrange("p (o h) -> p o h", o=1)
                                     .to_broadcast([128, NEG, H]),
                        in1=g[:], op=mult)
                    red1 = fp.tile([128, NEG, H // CH], BF16, tag="red1")
                    with nc.allow_low_precision(
                            reason="bf16 partial sums of 32 products; "
                                   "validated: final loss rel err ~1e-4"):
                        nc.vector.tensor_reduce(
                            out=red1[:],
                            in_=prod[:].rearrange("p n (c k) -> p n c k",
                                                  k=CH),
                            axis=X, op=add)
                    negf = sm.tile([128, NEG], F32, tag="negf")
                    nc.vector.tensor_reduce(
                        out=negf[:], in_=red1[:], axis=X, op=add)
                    nc.vector.tensor_scalar_mul(
                        out=logits[:, t * K + 1:(t + 1) * K], in0=negf[:],
                        scalar1=inv_t)
                elif not no_dot:
                    koff = ACT_DOTS if g is not None else 0
                    ndve = NEG - koff
                    for j in range(ndve):
                        c = t * K + 1 + j
                        in1 = g[:, j, :] if g is not None else pred_t[:]
                        nc.vector.scalar_tensor_tensor(
                            out=scr[:], in0=pred_t[:], scalar=inv_t,
                            in1=in1, op0=mult, op1=mult,
                            accum_out=logits[:, c:c + 1],
                        )
                    if koff:
                        sqs = sm.tile([128, koff], F32, tag="sqs")
                        for jj in range(koff):
                            j = ndve + jj
                            su = sup.tile([128, H], BF16, tag="su")
                            nc.vector.tensor_add(
                                out=su[:], in0=pred_t[:], in1=g[:, j, :])
                            scs = sap.tile([128, H], BF16, tag="scs")
                            nc.scalar.activation(
                                out=scs[:], in_=su[:], func=AF.Square,
                                accum_out=sqs[:, jj:jj + 1])
                else:
                    nc.vector.memset(logits[:, t * K + 1:(t + 1) * K], 1.0)

                # row norms on the Scalar engine (Square + accumulate);
                # pred.pos dot on DVE (scalar_tensor_tensor, fp32 accum).
                scra = sap.tile([128, H], BF16, tag="scra")
                nc.scalar.activation(
                    out=scra[:], in_=pred_t[:], func=AF.Square,
                    accum_out=pn2s[:, t:t + 1])
                scrb = sap.tile([128, H], BF16, tag="scrb")
                nc.scalar.activation(
                    out=scrb[:], in_=pos_t[:], func=AF.Square,
                    accum_out=qn2s[:, t:t + 1])
                nc.vector.scalar_tensor_tensor(
                    out=scr[:], in0=pred_t[:], scalar=inv_t, in1=pos_t[:],
                    op0=mult, op1=mult, accum_out=ppds[:, t:t + 1])

                if (not fused) and (not no_dot) and g is not None and ACT_DOTS:
                    # assemble the ACT-identity dots now that pn2 is ready:
                    # logit_j = (|p+g_j|^2 - |g_j|^2 - |p|^2) / (2T),
                    # with |g_j|^2 precomputed on host and gathered per row.
                    koff = ACT_DOTS
                    ndve = NEG - koff
                    cmb = sm.tile([128, koff], F32, tag="cmb")
                    nc.vector.tensor_sub(
                        out=cmb[:], in0=sqs[:],
                        in1=gn2_t[:, t * NEG + ndve:(t + 1) * NEG])
                    nc.vector.tensor_sub(
                        out=cmb[:], in0=cmb[:],
                        in1=pn2s[:, t:t + 1].to_broadcast([128, koff]))
                    nc.vector.tensor_scalar_mul(
                        out=logits[:, t * K + 1 + ndve:(t + 1) * K],
                        in0=cmb[:], scalar1=0.5 * inv_t)

            # Batched positive-logit epilogue over all NT tiles:
            # logits[:, t*K] = (ppd/T) / sqrt(pn2*qn2)
            l3 = logits[:].rearrange("p (t k) -> p t k", k=K)
            nrm = pers.tile([128, NT], F32, tag="nrm")
            nc.vector.tensor_mul(out=nrm[:], in0=pn2s[:], in1=qn2s[:])
            nc.scalar.activation(out=nrm[:], in_=nrm[:], func=AF.Sqrt)
            rn = pers.tile([128, NT], F32, tag="rn")
            nc.vector.reciprocal(out=rn[:], in_=nrm[:])
            nc.vector.tensor_mul(
                out=l3[:, :, 0:1],
                in0=ppds[:].rearrange("p (t o) -> p t o", o=1),
                in1=rn[:].rearrange("p (t o) -> p t o", o=1))

            # Batched logsumexp over all NT tiles at once.
            m = pers.tile([128, NT, 1], F32, tag="m")
            nc.vector.reduce_max(out=m[:], in_=l3, axis=X)
            sh = pers.tile([128, NT, K], F32, tag="sh")
            nc.vector.tensor_sub(out=sh[:], in0=l3, in1=m[:].to_broadcast([128, NT, K]))
            eh = pers.tile([128, NT, K], F32, tag="eh")
            nc.scalar.activation(out=eh[:], in_=sh[:], func=AF.Exp)
            ss = pers.tile([128, NT, 1], F32, tag="ss")
            nc.vector.reduce_sum(out=ss[:], in_=eh[:], axis=X)
            nc.scalar.activation(out=ss[:], in_=ss[:], func=AF.Ln)
            outt = pers.tile([128, NT], F32, tag="outt")
            nc.vector.tensor_add(out=outt[:], in0=m[:, :, 0], in1=ss[:, :, 0])
            nc.vector.tensor_sub(out=outt[:], in0=outt[:], in1=l3[:, :, 0])
            nc.sync.dma_start(out=out_d.ap(), in_=outt[:])

    nc.compile()
    return nc


def _prep_in_maps(encoder_outputs, event_embeddings, neg_indices):
    enc = np.asarray(encoder_outputs, dtype=np.float32)
    ev = np.asarray(event_embeddings, dtype=np.float32)
    ni = np.asarray(neg_indices)
    bf = mybir.dt.np(BF16)

    b_ids = np.arange(B, dtype=ni.dtype)[:, None, None]
    gidx = (ni + S * (ni >= b_ids * S).astype(ni.dtype)).astype(np.int32)

    ev_flat = np.ascontiguousarray(ev.reshape(B * S, H)).astype(bf)
    # |e|^2 per event row, from the same bf16-cast table the device dots
    # against (pure function of the inputs, like the index remap).
    ev_n2 = (ev_flat.astype(np.float32) ** 2).sum(-1)  # [B*S]

    in_maps = []
    for c in range(NCORES):
        bs = slice(c * BPC, (c + 1) * BPC)
        pred = enc[bs, :-1, :].reshape(R, H)
        pos = ev[bs, 1:, :].reshape(R, H)
        pred_p = np.ones((RP, H), np.float32)
        pred_p[:R] = pred
        pos_p = np.ones((RP, H), np.float32)
        pos_p[:R] = pos
        idx = np.zeros((RP, NEG), np.int32)
        idx[:R] = gidx[bs].reshape(R, NEG)
        # dma_gather index layout: per tile t the flat gather order is
        # i = j*128 + p; index i lives at partition i%16, column i//16.
        tiles = idx.reshape(NT, 128, NEG).transpose(0, 2, 1).reshape(NT, NIDX)
        arr = tiles.reshape(NT, ICOLS, 16).transpose(0, 2, 1)  # [NT, 16, ICOLS]
        band = arr.transpose(1, 0, 2).reshape(16, NT * ICOLS).astype(np.int16)
        dev = np.tile(band, (8, 1))  # replicate to all 8 Q7 partition groups
        # gathered |g_j|^2 per (row, neg): device layout [128, NT*NEG],
        # partition = row-within-tile, column = t*NEG + j
        gn2 = ev_n2[idx]  # [RP, NEG]
        gn2_dev = np.ascontiguousarray(
            gn2.reshape(NT, 128, NEG).transpose(1, 0, 2)
        ).reshape(128, NT * NEG).astype(np.float32)
        in_maps.append({
            "events": ev_flat,
            "pred": pred_p.astype(bf),
            "pos": pos_p.astype(bf),
            "idx": dev,
            "gn2": gn2_dev,
        })
    return in_maps


def _reduce_loss(results) -> np.float32:
    total = 0.0
    for c in range(NCORES):
        lr = np.asarray(results[c]["loss"], dtype=np.float64)  # [128, NT]
        rows = lr.T.reshape(RP)[:R]
        total += rows.sum()
    return np.float32(total / (B * (S - 1)))


_NC_CACHE: dict = {}


def kernel(encoder_outputs, event_embeddings, neg_indices, temperature):
    temp = float(np.asarray(temperature))
    nc = _NC_CACHE.get(temp)
    if nc is None:
        nc = _build(temp)
        _NC_CACHE[temp] = nc
    in_maps = _prep_in_maps(encoder_outputs, event_embeddings, neg_indices)
    res = run_bass_kernel_spmd(nc, in_maps, core_ids=list(range(NCORES)))
    return _reduce_loss(res.results)
